# revision 17
# baseline (speedup 1.0000x reference)
"""Trainium2 Bass kernel for nn_DC_CRD_85779086836063 (gnn_message_passing).

Reference math (B,C,H,W = 32,64,128,128):
    wvec = mean(x, (2,3))                          # [B, C]
    diff = wvec[:,:,None] - wvec[:,None,:]         # [B, C, C]
    e = exp(-diff); T = |1 - e/(1+e)| - 1          # = sigmoid(diff) - 1
    A = 0.5*(T + T^T) * theta                      # sigmoid(d)+sigmoid(-d) = 1
                                                   # => T + T^T = -1 (exactly)
                                                   # => A = -0.5 * theta  (data-independent)
    H = relu(A @ x_flat)                           # [B, C, HW]
    out = (W_lin @ H)^T + b_lin  reshaped raw [HW,C] -> [C,H,W]

So per batch: out[b] (as [HW, C]) = (W_lin @ relu(-0.5 theta @ x[b]))^T + b_lin.

Sharding: pure data parallel, batch dim 32 -> 4 per core across 8 cores;
theta/W_lin/b_lin replicated.

Per-core dataflow (2-batch packing to fill 128 partitions, C=64):
    Ablk = blockdiag(-0.5 theta^T, -0.5 theta^T)   [128,128]  (lhsT of mm1)
    Wblk = blockdiag(W_lin^T, W_lin^T)             [128,128]  (lhsT of mm2)
    per chunk: one 2 MiB DMA loads x2 = [x[b0]; x[b1]] stacked [128, n]
    per 512-col subchunk:
      ps1 = Ablk.T @ x2          (PE)
      h   = relu(ps1)            (ACT)   PSUM->SBUF
      ps2 = Wblk.T @ h           (PE)
      g   = ps2 + bias_pp        (ACT/DVE alternating) PSUM->SBUF
      ps3 = transpose(g) x4      (PE, 128x128 blocks)  -> [n, (2b,c)]
      o   = copy(ps3)            (DVE/ACT alternating) PSUM->SBUF
    one 2 MiB DMA stores o -> out[b0/b1, n, c]  ([HW, C] == reference reshape)

Variants (BASS_VARIANT): "f32"  exact fp32 matmuls (4 cyc/col on PE)
                         "mm2r" second matmul float32r
                         "allr" both matmuls + transposes float32r
                                (x rounded to f32r on GpSimd; ~3e-4 rel err)
"""

import os
import sys

sys.path.insert(0, "/opt/trn_rl_repo")

import numpy as np

import concourse.bacc as bacc
import concourse.mybir as mybir
from concourse import tile
from concourse.bass_utils import run_bass_kernel_spmd
from concourse.masks import make_identity

dt = mybir.dt
AF = mybir.ActivationFunctionType

B, C, H, W = 32, 64, 128, 128
HW = H * W
NCORES = 8
BL = B // NCORES  # batches per core
PAIRS = BL // 2

DMACHUNK = 2048  # cols per DMA chunk (1 MiB per DMA)
SUB = 512  # cols per matmul / PSUM bank

VARIANT = os.environ.get("BASS_VARIANT", "allr")


def _build(variant: str):
    d1 = dt.float32r if variant == "allr" else dt.float32  # mm1 operands
    d2 = dt.float32r if variant in ("mm2r", "allr") else dt.float32  # mm2
    dtr = dt.float32r if variant == "allr" else dt.float32  # transpose path

    nc = bacc.Bacc("TRN2", target_bir_lowering=False, debug=False)

    x_d = nc.dram_tensor("x", [BL, C, HW], dt.float32, kind="ExternalInput")
    th_d = nc.dram_tensor("theta", [C, C], dt.float32, kind="ExternalInput")
    wl_d = nc.dram_tensor("W_lin", [C, C], dt.float32, kind="ExternalInput")
    bl_d = nc.dram_tensor("b_lin", [C], dt.float32, kind="ExternalInput")
    out_d = nc.dram_tensor("out", [BL, HW, C], dt.float32, kind="ExternalOutput")

    with tile.TileContext(nc) as tc:
        with (
            tc.tile_pool(name="const", bufs=1) as const,
            tc.tile_pool(name="xp", bufs=3) as xp,
            tc.tile_pool(name="xrp", bufs=3) as xrp,
            tc.tile_pool(name="hp", bufs=3) as hp,
            tc.tile_pool(name="gp", bufs=4) as gp,
            tc.tile_pool(name="op", bufs=4) as op_,
        ):
            # ---------------- constants ----------------
            psc_cm = tc.tile_pool(name="psc", bufs=1, space="PSUM")
            psc = psc_cm.__enter__()
            ident = const.tile([128, 128], dt.float32, tag="ident")
            make_identity(nc, ident[:])
            if dtr == dt.float32r:
                ident_t = const.tile([128, 128], dt.float32r, tag="ident_r")
                nc.vector.tensor_copy(ident_t[:], ident[:])
            else:
                ident_t = ident

            # block-diag(theta, theta) and block-diag(W_lin, W_lin) in SBUF
            thb = const.tile([128, 128], dt.float32, tag="thb")
            wlb = const.tile([128, 128], dt.float32, tag="wlb")
            nc.gpsimd.memset(thb[:], 0.0)
            nc.gpsimd.memset(wlb[:], 0.0)
            nc.sync.dma_start(thb[0:64, 0:64], th_d[:])
            nc.sync.dma_start(thb[64:128, 64:128], th_d[:])
            nc.sync.dma_start(wlb[0:64, 0:64], wl_d[:])
            nc.sync.dma_start(wlb[64:128, 64:128], wl_d[:])

            # transpose on PE: psT = blockdiag(theta^T, theta^T), etc.
            psT = psc.tile([128, 512], dt.float32, tag="psT")
            nc.tensor.transpose(psT[:, 0:128], thb[:], ident[:])
            nc.tensor.transpose(psT[:, 128:256], wlb[:], ident[:])

            # Ablk = -0.5 * blockdiag(theta^T, theta^T)  (lhsT of mm1)
            ablk = const.tile([128, 128], d1, tag="ablk")
            nc.scalar.activation(ablk[:], psT[:, 0:128], AF.Copy, scale=-0.5)
            # Wblk = blockdiag(W_lin^T, W_lin^T)  (lhsT of mm2)
            wblk = const.tile([128, 128], d2, tag="wblk")
            nc.vector.tensor_copy(wblk[:], psT[:, 128:256])

            # per-partition bias [b_lin; b_lin] for the (2b, c) partition dim
            bias_pp = const.tile([128, 1], dt.float32, tag="bias_pp")
            blv = bl_d[:].rearrange("(c one) -> c one", one=1)
            nc.sync.dma_start(bias_pp[0:64, :], blv)
            nc.sync.dma_start(bias_pp[64:128, :], blv)

            psc_cm.__exit__(None, None, None)
            ps1p_cm = tc.tile_pool(name="ps1p", bufs=4, space="PSUM")
            ps2p_cm = tc.tile_pool(name="ps2p", bufs=2, space="PSUM")
            ps3p_cm = tc.tile_pool(name="ps3p", bufs=2, space="PSUM")
            ps1p = ps1p_cm.__enter__()
            ps2p = ps2p_cm.__enter__()
            ps3p = ps3p_cm.__enter__()

            # ---------------- main loop ----------------
            phase = 0
            CHUNKS = [512, 512, 1024] + [2048] * 6 + [1024, 512, 512]
            assert sum(CHUNKS) == HW
            for pair in range(PAIRS):
                b0 = 2 * pair
                n0 = 0
                for DMACHUNK in CHUNKS:
                    # one DMA, both batches, all 128 partitions (2D <-> 2D)
                    x2 = xp.tile([128, DMACHUNK], dt.float32, tag="x2")
                    xsrc = x_d[:].rearrange("b c n -> (b c) n")
                    nc.sync.dma_start(
                        x2[:], xsrc[b0 * C : (b0 + 2) * C, n0 : n0 + DMACHUNK]
                    )
                    if d1 == dt.float32r:
                        x2u = xrp.tile([128, DMACHUNK], dt.float32r, tag="x2r")
                        nc.vector.tensor_copy(x2u[:], x2[:])
                    else:
                        x2u = x2
                    # h holds the whole chunk, column-permuted so that transpose
                    # block t's partition p carries output row 32*p + t. Then
                    # each partition's (t, c) free span is 32 consecutive DRAM
                    # rows -> 8 KiB contiguous output DMA runs.
                    h = hp.tile([128, DMACHUNK], d2, tag="h")
                    R = DMACHUNK // 128  # output rows per partition
                    WW = SUB // R  # w-window per sub
                    hv = h[:].rearrange("p (t q) -> p t q", t=R)
                    o_b0 = op_.tile([128, DMACHUNK // 2], dt.float32, tag="o_b0")
                    o_b1 = op_.tile([128, DMACHUNK // 2], dt.float32, tag="o_b1")
                    NSUB = DMACHUNK // SUB
                    for s in range(NSUB):
                        xs = x2u[:, s * SUB : (s + 1) * SUB]
                        ps1 = ps1p.tile([128, SUB], dt.float32, tag="ps1")
                        nc.tensor.matmul(ps1[:], ablk[:], xs, start=True, stop=True)
                        # relu + permuted scatter: dst f = 128*(j%R) + WW*s + j//R
                        ps1v = ps1[:].rearrange("p (a r) -> p r a", r=R)
                        nc.scalar.activation(
                            hv[:, :, WW * s : WW * (s + 1)], ps1v, AF.Relu
                        )
                    for s in range(NSUB):
                        hs = h[:, s * SUB : (s + 1) * SUB]
                        ps2 = ps2p.tile([128, SUB], dt.float32, tag="ps2")
                        nc.tensor.matmul(ps2[:], wblk[:], hs, start=True, stop=True)
                        # bias-add + PSUM->SBUF (ACT)
                        g = gp.tile([128, SUB], dtr, tag="g")
                        nc.scalar.activation(g[:], ps2[:], AF.Identity, bias=bias_pp[:])
                        # transpose 128x128 blocks: [(2b,c), f] -> [row, (2b,c)]
                        # as NORMAL matmuls (lhsT.T @ I) so the PE HAM clock
                        # monitor sees matmul activity (transpose-mode doesn't
                        # count -> K=4/8 throttle tails).
                        ps3 = ps3p.tile([128, SUB], dt.float32, tag="ps3")
                        for k in range(SUB // 128):
                            nc.tensor.matmul(
                                ps3[:, k * 128 : (k + 1) * 128],
                                g[:, k * 128 : (k + 1) * 128],
                                ident_t[:],
                                start=True,
                                stop=True,
                            )
                        # PSUM->SBUF staging per batch so the output DMA is
                        # contiguous on BOTH sides (DVE)
                        p3v = ps3[:].rearrange("p (t c2) -> p t c2", c2=128)
                        for bi, ob in enumerate((o_b0, o_b1)):
                            nc.vector.tensor_copy(
                                ob[:].rearrange("p (s t c) -> p s t c", t=4, c=64)[
                                    :, s
                                ],
                                p3v[:, :, bi * 64 : (bi + 1) * 64],
                            )
                    # DMA out per batch: pure [128 x 8 KiB] contiguous block
                    # copy; partition p owns DRAM rows [32p, 32p+32) of this
                    # chunk. SWDGE (gpsimd) keeps the Sync queue for loads.
                    for bi, ob in enumerate((o_b0, o_b1)):
                        dd = out_d[b0 + bi, n0 : n0 + DMACHUNK, :].rearrange(
                            "(p tc) c -> p (tc c)", p=128
                        )
                        nc.gpsimd.dma_start(dd, ob[:])
                    n0 += DMACHUNK
            ps3p_cm.__exit__(None, None, None)
            ps2p_cm.__exit__(None, None, None)
            ps1p_cm.__exit__(None, None, None)

    nc.compile()
    return nc


def _ensure_ntff_hook():
    """Register the axon NTFF profile hook (profiling only; best-effort).

    The agent image's ``antenv`` lacks ``axon_hooks``, so ``trace=True`` in
    ``run_bass_kernel_spmd`` would ImportError. Recreate the module with the
    same ctypes hook ``trn_agent_boot.trn_boot`` would have registered.
    """
    import contextlib
    import ctypes
    import types

    if "antenv.axon_hooks" in sys.modules:
        return
    so_path = "/opt/axon/libaxon_pjrt.so"
    try:
        lib = ctypes.CDLL(so_path)
        lib.axon_start_nrt_profile.argtypes = [
            ctypes.POINTER(ctypes.c_int64),
            ctypes.c_size_t,
        ]
        lib.axon_start_nrt_profile.restype = ctypes.c_int64
        lib.axon_stop_nrt_profile.argtypes = [ctypes.c_char_p]
        lib.axon_stop_nrt_profile.restype = ctypes.c_int64
    except (OSError, AttributeError):
        lib = None

    @contextlib.contextmanager
    def _hook(output_dir, device_ids):
        import jax

        jax.devices()
        if device_ids:
            ids = (ctypes.c_int64 * len(device_ids))(*device_ids)
            rc = lib.axon_start_nrt_profile(ids, len(device_ids))
        else:
            rc = lib.axon_start_nrt_profile(None, 0)
        if rc != 0:
            raise RuntimeError(f"axon_start_nrt_profile rc={rc}")
        try:
            yield
        finally:
            n = lib.axon_stop_nrt_profile(str(output_dir).encode())
            print(f"ntff profile: {n} file(s) written to {output_dir}")

    hook = _hook if lib is not None else None
    mod = types.ModuleType("antenv.axon_hooks")
    mod.get_axon_ntff_profile_hook = lambda: hook
    mod.set_axon_ntff_profile_hook = lambda h: None
    sys.modules["antenv.axon_hooks"] = mod


_NC_CACHE = {}


def _get_nc(variant: str):
    if variant not in _NC_CACHE:
        _NC_CACHE[variant] = _build(variant)
    return _NC_CACHE[variant]


def _run(inputs: dict, trace: bool = False, variant: str | None = None):
    variant = variant or VARIANT
    if trace:
        _ensure_ntff_hook()
    nc = _get_nc(variant)
    x = np.ascontiguousarray(inputs["x"], dtype=np.float32)
    theta = np.ascontiguousarray(inputs["theta"], dtype=np.float32)
    w_lin = np.ascontiguousarray(inputs["W_lin"], dtype=np.float32)
    b_lin = np.ascontiguousarray(inputs["b_lin"], dtype=np.float32)
    in_maps = [
        {
            "x": np.ascontiguousarray(x[i * BL : (i + 1) * BL].reshape(BL, C, HW)),
            "theta": theta,
            "W_lin": w_lin,
            "b_lin": b_lin,
        }
        for i in range(NCORES)
    ]
    # Occasionally the first execution of a freshly-loaded NEFF fails with
    # NRT_EXEC_UNIT_UNRECOVERABLE; a retry on the recovered device succeeds.
    import time

    last_err = None
    for attempt in range(3):
        try:
            res = run_bass_kernel_spmd(
                nc, in_maps, core_ids=list(range(NCORES)), trace=trace
            )
            break
        except Exception as e:  # noqa: BLE001
            last_err = e
            time.sleep(10 * (attempt + 1))
    else:
        raise last_err
    shards = [r["out"].reshape(BL, C, H, W) for r in res.results]
    return np.concatenate(shards, axis=0), res


def kernel(x, theta, W_lin, b_lin):
    out, _ = _run({"x": x, "theta": theta, "W_lin": W_lin, "b_lin": b_lin})
    return out


# revision 18
# speedup vs baseline: 1.1243x; 1.1243x over previous
"""Trainium2 Bass kernel for nn_DC_CRD_85779086836063 (gnn_message_passing).

Reference math (B,C,H,W = 32,64,128,128):
    wvec = mean(x, (2,3))                          # [B, C]
    diff = wvec[:,:,None] - wvec[:,None,:]         # [B, C, C]
    e = exp(-diff); T = |1 - e/(1+e)| - 1          # = sigmoid(diff) - 1
    A = 0.5*(T + T^T) * theta                      # sigmoid(d)+sigmoid(-d) = 1
                                                   # => T + T^T = -1 (exactly)
                                                   # => A = -0.5 * theta  (data-independent)
    H = relu(A @ x_flat)                           # [B, C, HW]
    out = (W_lin @ H)^T + b_lin  reshaped raw [HW,C] -> [C,H,W]

So per batch: out[b] (as [HW, C]) = (W_lin @ relu(-0.5 theta @ x[b]))^T + b_lin.

Sharding: pure data parallel, batch dim 32 -> 4 per core across 8 cores;
theta/W_lin/b_lin replicated.

Per-core dataflow (2-batch packing to fill 128 partitions, C=64):
    Ablk = blockdiag(-0.5 theta^T, -0.5 theta^T)   [128,128]  (lhsT of mm1)
    Wblk = blockdiag(W_lin^T, W_lin^T)             [128,128]  (lhsT of mm2)
    per chunk: one 2 MiB DMA loads x2 = [x[b0]; x[b1]] stacked [128, n]
    per 512-col subchunk:
      ps1 = Ablk.T @ x2          (PE)
      h   = relu(ps1)            (ACT)   PSUM->SBUF
      ps2 = Wblk.T @ h           (PE)
      g   = ps2 + bias_pp        (ACT/DVE alternating) PSUM->SBUF
      ps3 = transpose(g) x4      (PE, 128x128 blocks)  -> [n, (2b,c)]
      o   = copy(ps3)            (DVE/ACT alternating) PSUM->SBUF
    one 2 MiB DMA stores o -> out[b0/b1, n, c]  ([HW, C] == reference reshape)

Variants (BASS_VARIANT): "f32"  exact fp32 matmuls (4 cyc/col on PE)
                         "mm2r" second matmul float32r
                         "allr" both matmuls + transposes float32r
                                (x rounded to f32r on GpSimd; ~3e-4 rel err)
"""

import os
import sys

sys.path.insert(0, "/opt/trn_rl_repo")

import numpy as np

import concourse.bacc as bacc
import concourse.mybir as mybir
from concourse import tile
from concourse.bass_utils import run_bass_kernel_spmd
from concourse.masks import make_identity

dt = mybir.dt
AF = mybir.ActivationFunctionType

B, C, H, W = 32, 64, 128, 128
HW = H * W
NCORES = 8
BL = B // NCORES  # batches per core
PAIRS = BL // 2

DMACHUNK = 2048  # cols per DMA chunk (1 MiB per DMA)
SUB = 512  # cols per matmul / PSUM bank

VARIANT = os.environ.get("BASS_VARIANT", "allr")


def _build(variant: str):
    d1 = dt.float32r if variant == "allr" else dt.float32  # mm1 operands
    d2 = dt.float32r if variant in ("mm2r", "allr") else dt.float32  # mm2
    dtr = dt.float32r if variant == "allr" else dt.float32  # transpose path

    nc = bacc.Bacc("TRN2", target_bir_lowering=False, debug=False)

    x_d = nc.dram_tensor("x", [BL, C, HW], dt.float32, kind="ExternalInput")
    th_d = nc.dram_tensor("theta", [C, C], dt.float32, kind="ExternalInput")
    wl_d = nc.dram_tensor("W_lin", [C, C], dt.float32, kind="ExternalInput")
    bl_d = nc.dram_tensor("b_lin", [C], dt.float32, kind="ExternalInput")
    out_d = nc.dram_tensor("out", [BL, HW, C], dt.float32, kind="ExternalOutput")

    with tile.TileContext(nc) as tc:
        with (
            tc.tile_pool(name="const", bufs=1) as const,
            tc.tile_pool(name="xp", bufs=3) as xp,
            tc.tile_pool(name="xrp", bufs=3) as xrp,
            tc.tile_pool(name="hp", bufs=3) as hp,
            tc.tile_pool(name="gp", bufs=4) as gp,
            tc.tile_pool(name="op", bufs=4) as op_,
        ):
            # ---------------- constants ----------------
            psc_cm = tc.tile_pool(name="psc", bufs=1, space="PSUM")
            psc = psc_cm.__enter__()
            ident = const.tile([128, 128], dt.float32, tag="ident")
            make_identity(nc, ident[:])
            if dtr == dt.float32r:
                ident_t = const.tile([128, 128], dt.float32r, tag="ident_r")
                nc.vector.tensor_copy(ident_t[:], ident[:])
            else:
                ident_t = ident

            # block-diag(theta, theta) and block-diag(W_lin, W_lin) in SBUF
            thb = const.tile([128, 128], dt.float32, tag="thb")
            wlb = const.tile([128, 128], dt.float32, tag="wlb")
            nc.gpsimd.memset(thb[:], 0.0)
            nc.gpsimd.memset(wlb[:], 0.0)
            nc.sync.dma_start(thb[0:64, 0:64], th_d[:])
            nc.sync.dma_start(thb[64:128, 64:128], th_d[:])
            nc.sync.dma_start(wlb[0:64, 0:64], wl_d[:])
            nc.sync.dma_start(wlb[64:128, 64:128], wl_d[:])

            # transpose on PE: psT = blockdiag(theta^T, theta^T), etc.
            psT = psc.tile([128, 512], dt.float32, tag="psT")
            nc.tensor.transpose(psT[:, 0:128], thb[:], ident[:])
            nc.tensor.transpose(psT[:, 128:256], wlb[:], ident[:])

            # Ablk = -0.5 * blockdiag(theta^T, theta^T)  (lhsT of mm1)
            ablk = const.tile([128, 128], d1, tag="ablk")
            nc.scalar.activation(ablk[:], psT[:, 0:128], AF.Copy, scale=-0.5)
            # Wblk = blockdiag(W_lin^T, W_lin^T)  (lhsT of mm2)
            wblk = const.tile([128, 128], d2, tag="wblk")
            nc.vector.tensor_copy(wblk[:], psT[:, 128:256])

            # per-partition bias [b_lin; b_lin] for the (2b, c) partition dim
            bias_pp = const.tile([128, 1], dt.float32, tag="bias_pp")
            blv = bl_d[:].rearrange("(c one) -> c one", one=1)
            nc.sync.dma_start(bias_pp[0:64, :], blv)
            nc.sync.dma_start(bias_pp[64:128, :], blv)

            psc_cm.__exit__(None, None, None)
            ps1p_cm = tc.tile_pool(name="ps1p", bufs=4, space="PSUM")
            ps2p_cm = tc.tile_pool(name="ps2p", bufs=2, space="PSUM")
            ps3p_cm = tc.tile_pool(name="ps3p", bufs=2, space="PSUM")
            ps1p = ps1p_cm.__enter__()
            ps2p = ps2p_cm.__enter__()
            ps3p = ps3p_cm.__enter__()

            # ---------------- main loop ----------------
            phase = 0
            # taper only at global head and tail, keep pair junction coarse
            HEAD = [512, 512, 1024] + [2048] * 7
            TAIL = [2048] * 7 + [1024, 512, 512]
            assert sum(HEAD) == HW and sum(TAIL) == HW
            for pair in range(PAIRS):
                b0 = 2 * pair
                n0 = 0
                for DMACHUNK in (HEAD if pair == 0 else TAIL):
                    # one DMA, both batches, all 128 partitions (2D <-> 2D)
                    x2 = xp.tile([128, DMACHUNK], dt.float32, tag="x2")
                    xsrc = x_d[:].rearrange("b c n -> (b c) n")
                    nc.sync.dma_start(
                        x2[:], xsrc[b0 * C : (b0 + 2) * C, n0 : n0 + DMACHUNK]
                    )
                    if d1 == dt.float32r:
                        x2u = xrp.tile([128, DMACHUNK], dt.float32r, tag="x2r")
                        nc.vector.tensor_copy(x2u[:], x2[:])
                    else:
                        x2u = x2
                    # h holds the whole chunk, column-permuted so that transpose
                    # block t's partition p carries output row 32*p + t. Then
                    # each partition's (t, c) free span is 32 consecutive DRAM
                    # rows -> 8 KiB contiguous output DMA runs.
                    h = hp.tile([128, DMACHUNK], d2, tag="h")
                    R = DMACHUNK // 128  # output rows per partition
                    WW = SUB // R  # w-window per sub
                    hv = h[:].rearrange("p (t q) -> p t q", t=R)
                    o_b0 = op_.tile([128, DMACHUNK // 2], dt.float32, tag="o_b0")
                    o_b1 = op_.tile([128, DMACHUNK // 2], dt.float32, tag="o_b1")
                    NSUB = DMACHUNK // SUB
                    for s in range(NSUB):
                        xs = x2u[:, s * SUB : (s + 1) * SUB]
                        ps1 = ps1p.tile([128, SUB], dt.float32, tag="ps1")
                        nc.tensor.matmul(ps1[:], ablk[:], xs, start=True, stop=True)
                        # relu + permuted scatter: dst f = 128*(j%R) + WW*s + j//R
                        ps1v = ps1[:].rearrange("p (a r) -> p r a", r=R)
                        nc.scalar.activation(
                            hv[:, :, WW * s : WW * (s + 1)], ps1v, AF.Relu
                        )
                    for s in range(NSUB):
                        hs = h[:, s * SUB : (s + 1) * SUB]
                        ps2 = ps2p.tile([128, SUB], dt.float32, tag="ps2")
                        nc.tensor.matmul(ps2[:], wblk[:], hs, start=True, stop=True)
                        # bias-add + PSUM->SBUF (ACT)
                        g = gp.tile([128, SUB], dtr, tag="g")
                        nc.scalar.activation(g[:], ps2[:], AF.Identity, bias=bias_pp[:])
                        # transpose 128x128 blocks: [(2b,c), f] -> [row, (2b,c)]
                        # as NORMAL matmuls (lhsT.T @ I) so the PE HAM clock
                        # monitor sees matmul activity (transpose-mode doesn't
                        # count -> K=4/8 throttle tails).
                        ps3 = ps3p.tile([128, SUB], dt.float32, tag="ps3")
                        for k in range(SUB // 128):
                            nc.tensor.matmul(
                                ps3[:, k * 128 : (k + 1) * 128],
                                g[:, k * 128 : (k + 1) * 128],
                                ident_t[:],
                                start=True,
                                stop=True,
                            )
                        # PSUM->SBUF staging per batch so the output DMA is
                        # contiguous on BOTH sides (DVE)
                        p3v = ps3[:].rearrange("p (t c2) -> p t c2", c2=128)
                        for bi, ob in enumerate((o_b0, o_b1)):
                            nc.vector.tensor_copy(
                                ob[:].rearrange("p (s t c) -> p s t c", t=4, c=64)[
                                    :, s
                                ],
                                p3v[:, :, bi * 64 : (bi + 1) * 64],
                            )
                    # DMA out per batch: pure [128 x 8 KiB] contiguous block
                    # copy; partition p owns DRAM rows [32p, 32p+32) of this
                    # chunk. SWDGE (gpsimd) keeps the Sync queue for loads.
                    for bi, ob in enumerate((o_b0, o_b1)):
                        dd = out_d[b0 + bi, n0 : n0 + DMACHUNK, :].rearrange(
                            "(p tc) c -> p (tc c)", p=128
                        )
                        nc.gpsimd.dma_start(dd, ob[:])
                    n0 += DMACHUNK
            ps3p_cm.__exit__(None, None, None)
            ps2p_cm.__exit__(None, None, None)
            ps1p_cm.__exit__(None, None, None)

    nc.compile()
    return nc


def _ensure_ntff_hook():
    """Register the axon NTFF profile hook (profiling only; best-effort).

    The agent image's ``antenv`` lacks ``axon_hooks``, so ``trace=True`` in
    ``run_bass_kernel_spmd`` would ImportError. Recreate the module with the
    same ctypes hook ``trn_agent_boot.trn_boot`` would have registered.
    """
    import contextlib
    import ctypes
    import types

    if "antenv.axon_hooks" in sys.modules:
        return
    so_path = "/opt/axon/libaxon_pjrt.so"
    try:
        lib = ctypes.CDLL(so_path)
        lib.axon_start_nrt_profile.argtypes = [
            ctypes.POINTER(ctypes.c_int64),
            ctypes.c_size_t,
        ]
        lib.axon_start_nrt_profile.restype = ctypes.c_int64
        lib.axon_stop_nrt_profile.argtypes = [ctypes.c_char_p]
        lib.axon_stop_nrt_profile.restype = ctypes.c_int64
    except (OSError, AttributeError):
        lib = None

    @contextlib.contextmanager
    def _hook(output_dir, device_ids):
        import jax

        jax.devices()
        if device_ids:
            ids = (ctypes.c_int64 * len(device_ids))(*device_ids)
            rc = lib.axon_start_nrt_profile(ids, len(device_ids))
        else:
            rc = lib.axon_start_nrt_profile(None, 0)
        if rc != 0:
            raise RuntimeError(f"axon_start_nrt_profile rc={rc}")
        try:
            yield
        finally:
            n = lib.axon_stop_nrt_profile(str(output_dir).encode())
            print(f"ntff profile: {n} file(s) written to {output_dir}")

    hook = _hook if lib is not None else None
    mod = types.ModuleType("antenv.axon_hooks")
    mod.get_axon_ntff_profile_hook = lambda: hook
    mod.set_axon_ntff_profile_hook = lambda h: None
    sys.modules["antenv.axon_hooks"] = mod


_NC_CACHE = {}


def _get_nc(variant: str):
    if variant not in _NC_CACHE:
        _NC_CACHE[variant] = _build(variant)
    return _NC_CACHE[variant]


def _run(inputs: dict, trace: bool = False, variant: str | None = None):
    variant = variant or VARIANT
    if trace:
        _ensure_ntff_hook()
    nc = _get_nc(variant)
    x = np.ascontiguousarray(inputs["x"], dtype=np.float32)
    theta = np.ascontiguousarray(inputs["theta"], dtype=np.float32)
    w_lin = np.ascontiguousarray(inputs["W_lin"], dtype=np.float32)
    b_lin = np.ascontiguousarray(inputs["b_lin"], dtype=np.float32)
    in_maps = [
        {
            "x": np.ascontiguousarray(x[i * BL : (i + 1) * BL].reshape(BL, C, HW)),
            "theta": theta,
            "W_lin": w_lin,
            "b_lin": b_lin,
        }
        for i in range(NCORES)
    ]
    # Occasionally the first execution of a freshly-loaded NEFF fails with
    # NRT_EXEC_UNIT_UNRECOVERABLE; a retry on the recovered device succeeds.
    import time

    last_err = None
    for attempt in range(3):
        try:
            res = run_bass_kernel_spmd(
                nc, in_maps, core_ids=list(range(NCORES)), trace=trace
            )
            break
        except Exception as e:  # noqa: BLE001
            last_err = e
            time.sleep(10 * (attempt + 1))
    else:
        raise last_err
    shards = [r["out"].reshape(BL, C, H, W) for r in res.results]
    return np.concatenate(shards, axis=0), res


def kernel(x, theta, W_lin, b_lin):
    out, _ = _run({"x": x, "theta": theta, "W_lin": W_lin, "b_lin": b_lin})
    return out
